# revision 1
# baseline (speedup 1.0000x reference)
"""Causal attention with key padding for Trainium2, sharded over 8 NeuronCores.

Contract: kernel(**inputs) takes the FULL inputs (q, k, v, att_mask, pad_mask)
as numpy arrays and returns the FULL [B, H, L, D] output.

Strategy (v4):
  - Shard the 64 (batch, head) units across 8 cores: core c gets units
    [8c, 8c+8), so each core sees a single batch's pad mask.
  - Host pre-transposes Q and K to [unit, D, L] (bf16); V is packed with a
    ones-column (for softmax denominators) and padded key rows zeroed, so
    padding costs nothing on device.
  - Per head and per 512-wide query block: S^T chunks (keys on partitions)
    via bf16 matmuls 2x row-packed (contract D=64) for full chunks;
    crossing (diagonal) chunks computed only on their live query columns
    (unpacked: mixed-width concurrent row groups fault the PE).
  - exp() is split across TWO engines, load-balanced at emission time with
    a ScalarE preference: ScalarE runs the exact spline exp; VectorE runs a
    Schraudolph-style exp (single tensor_scalar f32->int16: i = round(x*A +
    B), bitcast to bf16).  Causal boundary = one [128,128] triangle
    multiply per crossing chunk.  P~ @ [V|1] accumulates unnormalized
    outputs + denominators in PSUM.
  - The [65, 512] accumulator is copied to SBUF (engine load-balanced) and
    DMA'd out.  Normalization (num/den) and the final [D, L] -> [L, D]
    transpose happen on the host.
"""

import numpy as np

N_CORES = 8
KC = 128          # key-chunk (partition) size
QB = 512          # query-block width
MAX_BATCH = 3     # S^T chunks per batch (<= 3 PSUM banks)

# Schraudolph exp constants (bf16 via int16, round-to-nearest conversion):
# exp(s*scale) ~ bitcast_bf16(int16(round(s*EXP_A + EXP_B)))
_LOG2E = 1.4426950408889634
_EXP_C = 0.04305  # centers the log-linear interpolation error


# --------------------------------------------------------------------------
# numpy fallback (exact reference math) -- only used if the input masks do
# not match the causal + suffix-pad structure this kernel specializes to.
# --------------------------------------------------------------------------
def _reference_np(q, k, v, att_mask, pad_mask):
    B, H, L, D = q.shape
    scale = np.float32(1.0) / np.sqrt(np.float32(D))
    out = np.empty_like(q)
    for b in range(B):
        for h in range(H):
            att = (q[b, h] @ k[b, h].T) * scale
            att = att + att_mask[0, 0]
            att = np.where(pad_mask[b][None, :], -np.inf, att)
            att = att - att.max(axis=-1, keepdims=True)
            p = np.exp(att)
            p = p / p.sum(axis=-1, keepdims=True)
            out[b, h] = p @ v[b, h]
    return out


# --------------------------------------------------------------------------
# Bass program builder
# --------------------------------------------------------------------------
def _build_program(NH, L, D, skip):
    """Build the per-core SPMD Bass program.

    NH: heads per core.  L: sequence length.  D: head dim (<= 127, contract).
    skip: frozenset of fully-padded key chunks (never computed).
    """
    import os

    import concourse.bacc as bacc
    import concourse.mybir as mybir
    import concourse.tile as tile

    f32 = mybir.dt.float32
    bf16 = mybir.dt.bfloat16
    i16 = mybir.dt.int16
    NCH = L // KC
    NQB = L // QB
    CPB = QB // KC  # chunks spanning one query block (crossing chunks)
    scale = float(1.0 / np.sqrt(np.float32(D)))
    exp_a16 = float(128.0 * _LOG2E * scale)
    exp_b16 = float(128.0 * (127.0 - _EXP_C))
    BANK = 512  # fp32 elements per PSUM bank
    # emission-time engine-load skew: ScalarE (exact exp) is preferred until
    # it is this many ns ahead of VectorE (approx exp) -- accuracy knob
    ve_bias = float(os.environ.get("KVE_BIAS", "0"))
    cross_sc = os.environ.get("KCROSS_SC", "1") == "1"
    tri_gp = os.environ.get("KTRI_GP", "0") == "1"
    pv_lag = int(os.environ.get("KPV_LAG", "3"))
    sbuf_dup = os.environ.get("KSBUF_DUP", "1") == "1"
    out_bf16 = os.environ.get("KOUT_BF16", "1") == "1"

    nc = bacc.Bacc("TRN2", target_bir_lowering=False, debug=False)

    qt_d = nc.dram_tensor("qt", [NH, D, L], bf16, kind="ExternalInput")
    kt_d = nc.dram_tensor("kt", [NH, D, L], bf16, kind="ExternalInput")
    vo_d = nc.dram_tensor("vo", [NH, L, D + 1], bf16, kind="ExternalInput")
    tri_d = nc.dram_tensor("trimask", [KC, KC], bf16, kind="ExternalInput")
    out_d = nc.dram_tensor("out", [NH, NQB, D + 1, QB],
                           bf16 if out_bf16 else f32,
                           kind="ExternalOutput")

    load = {"sc": 0.0, "ve": 0.0}

    def route(cost_sc, cost_ve):
        if load["sc"] + cost_sc <= load["ve"] + cost_ve + ve_bias:
            load["sc"] += cost_sc
            return "sc"
        load["ve"] += cost_ve
        return "ve"

    with tile.TileContext(nc) as tc:
        with (
            tc.tile_pool(name="consts", bufs=1) as consts,
            tc.tile_pool(name="ktp", bufs=4) as ktp,
            tc.tile_pool(name="qtp", bufs=4) as qtp,
            tc.tile_pool(name="vop", bufs=4) as vop,
            tc.tile_pool(name="ptp", bufs=8) as ptp,
            tc.tile_pool(name="osb", bufs=4) as osb,
            tc.tile_pool(name="stg", bufs=2, space="PSUM") as stgp,
            tc.tile_pool(name="acc", bufs=2, space="PSUM") as accp,
        ):
            tri = consts.tile([KC, KC], bf16)

            # Warm-up with no DMA dependency: sustained PE activity makes
            # the clock gate grant full rate sooner, and a dummy exp pulls
            # the ACT table load off the critical path -- all while the
            # first head's tensors stream in.
            wsrc = consts.tile([KC, KC], bf16, tag="wsrc")
            wout = consts.tile([KC, 1], f32, tag="wout")
            nc.vector.memset(wsrc[:], 0.0)
            warm = stgp.tile([KC, MAX_BATCH * QB], f32, tag="stg")
            for i in range(60):
                nc.tensor.matmul(
                    out=warm[:, 0:KC], lhsT=wsrc[:], rhs=wsrc[:], start=True,
                    stop=True,
                )
                if i == 0:
                    nc.scalar.activation(
                        out=wout[:],
                        in_=warm[:, 0:1],
                        func=mybir.ActivationFunctionType.Exp,
                    )

            nc.sync.dma_start(out=tri[:], in_=tri_d[:])

            # Globally software-pipelined emission: PV for a batch is
            # emitted one batch later (so the PE never queues behind the
            # exp engines), and each query block's epilogue one further
            # batch later.
            pv_queue = []   # [(tick, acc, pt, items, vo_t, is_first, is_last)]
            epi_queue = []  # [(tick, h, qb, acc)]
            tick = [0]

            def emit_pv(job):
                _, acc, pt, items, vo_t, is_first, is_last = job
                for i, (kc, off, width) in enumerate(items):
                    qs = QB - width
                    nc.tensor.matmul(
                        out=acc[:, qs:QB],
                        lhsT=vo_t[:, kc, :],
                        rhs=pt[:, off : off + width].bitcast(bf16),
                        start=(is_first and i == 0),
                        stop=(is_last and i == len(items) - 1),
                    )

            def emit_epi(job):
                _, h, qb, acc = job
                o_t = osb.tile([D + 1, QB], bf16 if out_bf16 else f32)
                c_sc = (172 + QB) / 1.2
                c_ve = (120 + QB) / 0.96
                if route(c_sc, c_ve) == "sc":
                    nc.scalar.copy(out=o_t[:], in_=acc[:])
                else:
                    nc.vector.tensor_copy(out=o_t[:], in_=acc[:])
                nc.gpsimd.dma_start(out=out_d[h, qb], in_=o_t[:])

            def flush(drain=False):
                while epi_queue and (drain or epi_queue[0][0] < tick[0]):
                    emit_epi(epi_queue.pop(0))
                # PV trails its batch by pv_lag batches so the PE never
                # queues behind the exp engines (FIFO head-of-line)
                while pv_queue and (
                    drain or pv_queue[0][0] <= tick[0] - pv_lag
                ):
                    job = pv_queue.pop(0)
                    emit_pv(job)
                    acc = job[1]
                    if job[-1]:  # closed this query block
                        epi_queue.append(
                            (tick[0], job_h[id(acc)], job_qb[id(acc)], acc)
                        )

            job_h = {}
            job_qb = {}

            for h in range(NH):
                # K^T and Q^T are duplicated into both partition halves so
                # full-chunk QK^T matmuls run 2x row-packed (contract D=64)
                # via tile_position row groups.
                kt_t = ktp.tile([2 * D, L], bf16)
                qt_t = qtp.tile([2 * D, L], bf16)
                vo_t = vop.tile([KC, NCH, D + 1], bf16)
                # first query block's slices land first so head 0 can
                # start long before the full tensors arrive
                for lo, hi in ((0, QB), (QB, L)):
                    nc.sync.dma_start(
                        out=kt_t[0:D, lo:hi], in_=kt_d[h, :, lo:hi]
                    )
                    nc.sync.dma_start(
                        out=qt_t[0:D, lo:hi], in_=qt_d[h, :, lo:hi]
                    )
                    # partition-half duplicates: from SBUF (saves HBM) or
                    # a second DRAM fetch (shorter dependency chain)
                    nc.sync.dma_start(
                        out=kt_t[D : 2 * D, lo:hi],
                        in_=kt_t[0:D, lo:hi] if sbuf_dup else kt_d[h, :, lo:hi],
                    )
                    nc.sync.dma_start(
                        out=qt_t[D : 2 * D, lo:hi],
                        in_=qt_t[0:D, lo:hi] if sbuf_dup else qt_d[h, :, lo:hi],
                    )
                nc.gpsimd.dma_start(
                    out=vo_t[:],
                    in_=vo_d[h].rearrange("(c p) d -> p c d", p=KC),
                )

                qb_order = (
                    list(reversed(range(NQB))) if h == NH - 1 else range(NQB)
                )
                for qb in qb_order:
                    # batches: groups of full chunks, then one crossing
                    # batch (first-fit-decreasing into PSUM banks so matmul
                    # outputs stay bank-local and written columns form
                    # contiguous runs -- exp may only read written PSUM)
                    # the qb's first crossing chunk is full-width: fold it
                    # into the packed full batches (its triangle fixup is
                    # keyed on kc, not batch membership)
                    full = [kc for kc in range(CPB * qb) if kc not in skip]
                    if CPB * qb < NCH and CPB * qb not in skip:
                        full.append(CPB * qb)
                    batches = []
                    for i in range(0, len(full), MAX_BATCH):
                        batches.append(
                            [
                                (kc, QB * j, QB)
                                for j, kc in enumerate(full[i : i + MAX_BATCH])
                            ]
                        )
                    cross = sorted(
                        (
                            (QB - (kc - CPB * qb) * KC, kc)
                            for kc in range(CPB * qb + 1, min(CPB * (qb + 1), NCH))
                            if kc not in skip
                        ),
                        reverse=True,
                    )
                    bank_used = [0] * MAX_BATCH
                    crossing = []
                    for width, kc in cross:
                        b = next(
                            j for j, u in enumerate(bank_used)
                            if u + width <= BANK
                        )
                        crossing.append((kc, b * BANK + bank_used[b], width))
                        bank_used[b] += width
                    if crossing:
                        batches.append(sorted(crossing, key=lambda t: t[1]))

                    acc = accp.tile([D + 1, QB], f32)
                    job_h[id(acc)] = h
                    job_qb[id(acc)] = qb

                    for bi, items in enumerate(batches):
                        stg = stgp.tile([KC, MAX_BATCH * QB], f32, tag="stg")
                        # 2x row-group packing is only safe for equal-width
                        # matmuls (mixed-width concurrent row groups fault)
                        pack = all(it[2] == QB for it in items)
                        for i, (kc, off, width) in enumerate(items):
                            half = i % 2 if pack else 0
                            qlo = qb * QB + (QB - width)
                            nc.tensor.matmul(
                                out=stg[:, off : off + width],
                                lhsT=kt_t[
                                    half * D : (half + 1) * D,
                                    kc * KC : (kc + 1) * KC,
                                ],
                                rhs=qt_t[half * D : (half + 1) * D,
                                         qlo : (qb + 1) * QB],
                                start=True,
                                stop=True,
                            )
                        # contiguous written-column runs
                        runs = []
                        for _, off, width in items:
                            if runs and runs[-1][1] == off:
                                runs[-1][1] += width
                            else:
                                runs.append([off, off + width])
                        pt = ptp.tile([KC, MAX_BATCH * QB], i16)
                        c_sc = sum((172 + r1 - r0) / 1.2 for r0, r1 in runs)
                        c_ve = sum((120 + r1 - r0) / 0.96 for r0, r1 in runs)
                        is_cross = items[-1][0] >= CPB * qb
                        if cross_sc and is_cross:
                            load["sc"] += c_sc
                            eng = "sc"
                        else:
                            eng = route(c_sc, c_ve)
                        for r0, r1 in runs:
                            if eng == "sc":
                                nc.scalar.activation(
                                    out=pt[:, r0:r1].bitcast(bf16),
                                    in_=stg[:, r0:r1],
                                    func=mybir.ActivationFunctionType.Exp,
                                    scale=scale,
                                )
                            else:
                                nc.vector.tensor_scalar(
                                    out=pt[:, r0:r1],
                                    in0=stg[:, r0:r1],
                                    scalar1=exp_a16,
                                    scalar2=exp_b16,
                                    op0=mybir.AluOpType.mult,
                                    op1=mybir.AluOpType.add,
                                )
                        # causal boundary fixup on each crossing chunk's
                        # leading [128,128] block
                        for kc, off, width in items:
                            if kc >= CPB * qb:
                                pcol = pt[:, off : off + KC].bitcast(bf16)
                                if tri_gp:
                                    nc.gpsimd.tensor_mul(
                                        out=pcol, in0=pcol, in1=tri[:]
                                    )
                                else:
                                    nc.vector.tensor_mul(
                                        out=pcol, in0=pcol, in1=tri[:]
                                    )
                                    load["ve"] += (58 + KC / 2) / 0.96
                        flush()
                        pv_queue.append(
                            (tick[0], acc, pt, items, vo_t,
                             bi == 0, bi == len(batches) - 1)
                        )
                        tick[0] += 1
            flush(drain=True)
            flush(drain=True)
    if os.environ.get("KDEBUG_ROUTE"):
        print(f"route loads: sc={load['sc']:.0f}ns ve={load['ve']:.0f}ns "
              f"(ve_bias={ve_bias:.0f})")
    nc.finalize()
    return nc


# --------------------------------------------------------------------------
# host-side wrapper
# --------------------------------------------------------------------------
_PROG_CACHE = {}


def _get_program(NH, L, D, skip):
    key = (NH, L, D, skip)
    if key not in _PROG_CACHE:
        _PROG_CACHE[key] = _build_program(NH, L, D, skip)
    return _PROG_CACHE[key]


def _causal_ok(att_mask, L):
    if att_mask.shape != (1, 1, L, L):
        return False
    m = att_mask[0, 0]
    iu = np.triu_indices(L, 1)
    if not np.all(m[iu] == np.float32(-1e9)):
        return False
    il = np.tril_indices(L)
    return bool(np.all(m[il] == 0.0))


def kernel(q, k, v, att_mask, pad_mask):
    import ml_dtypes

    from concourse.bass_utils import run_bass_kernel_spmd

    B, H, L, D = q.shape
    U = B * H
    NQB = L // QB
    NCH = L // KC
    if (
        U % N_CORES != 0
        or L % QB != 0
        or D > KC - 1
        or not _causal_ok(att_mask, L)
    ):
        return _reference_np(q, k, v, att_mask, pad_mask)

    NH = U // N_CORES  # units per core

    pad = np.asarray(pad_mask, dtype=bool)          # [B, L]
    pad_u = np.repeat(pad, H, axis=0)               # [U, L]

    skip = frozenset(
        kc for kc in range(NCH)
        if np.all(pad_u[:, kc * KC : (kc + 1) * KC])
    )
    per_u_skip = [
        frozenset(
            kc for kc in range(NCH)
            if np.all(pad_u[u, kc * KC : (kc + 1) * KC])
        )
        for u in range(U)
    ]
    if 0 in skip or any(s != skip for s in per_u_skip):
        return _reference_np(q, k, v, att_mask, pad_mask)

    bf = ml_dtypes.bfloat16
    qf = np.ascontiguousarray(
        q.reshape(U, L, D).transpose(0, 2, 1)
    ).astype(bf)
    kf = np.ascontiguousarray(
        k.reshape(U, L, D).transpose(0, 2, 1)
    ).astype(bf)
    vo = np.empty((U, L, D + 1), dtype=np.float32)
    vo[:, :, 0:D] = v.reshape(U, L, D)
    vo[:, :, D] = 1.0
    vo[pad_u] = 0.0
    vo = vo.astype(bf)

    tri = (np.arange(KC)[None, :] >= np.arange(KC)[:, None]).astype(bf)

    in_maps = []
    for c in range(N_CORES):
        sl = slice(c * NH, (c + 1) * NH)
        in_maps.append(
            {"qt": qf[sl], "kt": kf[sl], "vo": vo[sl], "trimask": tri}
        )

    nc = _get_program(NH, L, D, skip)
    import os

    kwargs = {}
    if os.environ.get("BASS_KERNEL_PROFILE") == "1":
        kwargs = dict(trace=True, trace_cores=[0], stitch_traces=False)
    res = run_bass_kernel_spmd(nc, in_maps, list(range(N_CORES)), **kwargs)
    global LAST_RESULT
    LAST_RESULT = res
    raw = np.concatenate(
        [r["out"].astype(np.float32) for r in res.results], axis=0
    )
    # raw: [U, NQB, D+1, QB] unnormalized -- normalize + transpose on host
    num = raw[:, :, 0:D, :]
    den = raw[:, :, D : D + 1, :]
    out = (num / den).transpose(0, 1, 3, 2)        # [U, NQB, QB, D]
    out = np.ascontiguousarray(out).reshape(B, H, L, D)
    return out.astype(q.dtype, copy=False)


LAST_RESULT = None

